# Initial kernel scaffold
#
"""Trainium2 Bass kernel for nn_Attention_15899968929956.

Block-diagonal GNN message passing == dense per-system attention:
64 systems x 64 electrons, DIM=256, 8 heads x head_dim 32. Edges are all
intra-system pairs, so per (system, head):
  S'[j, i] = K[j] . Q[i] / sqrt(hd)           (j, i in [0, 64))
  P[j, i]  = exp(S') / sum_i' exp(S'[j, i'])  (softmax segmented by key j)
  attn[i]  = sum_j P[j, i] * V[j]
then out = LN2(h3 + silu(h3 @ W_mlp + b)), h3 = LN1(h + attn @ W_out).

Sharding: 8 systems (512 electrons) per NeuronCore, parameters replicated.
"""

import sys

if "/opt/trn_rl_repo" not in sys.path:
    sys.path.insert(0, "/opt/trn_rl_repo")

import numpy as np

N_SYS = 64
N_ELEC = 64
DIM = 256
HEADS = 8
HD = DIM // HEADS  # 32
EPS = 1e-6
NCORES = 8
SPC = N_SYS // NCORES      # systems per core = 8
R = SPC * N_ELEC           # rows per core = 512
NPAIR = SPC // 2           # system pairs per core = 4
NBLK = R // 128            # 128-row blocks per core = 4
SCALE = 1.0 / float(np.sqrt(HD))

# "f32" (exact) or "f32r" (reduced-precision multiplies on the big matmuls,
# full PE rate at N>=256 instead of 1/4 rate for fp32)
BIG_MM_DTYPE = "f32r"

_BUILD_CACHE: dict = {}


def _expected_edges():
    ii, jj = np.meshgrid(np.arange(N_ELEC), np.arange(N_ELEC), indexing="ij")
    offs = (np.arange(N_SYS) * N_ELEC)[:, None, None]
    ei = (offs + ii[None]).reshape(-1).astype(np.int32)
    ej = (offs + jj[None]).reshape(-1).astype(np.int32)
    return ei, ej


def _edges_are_blockdense(e_e_i, e_e_j):
    ei, ej = _expected_edges()
    a = np.asarray(e_e_i).ravel()
    b = np.asarray(e_e_j).ravel()
    if a.shape != ei.shape or b.shape != ej.shape:
        return False
    if np.array_equal(a, ei) and np.array_equal(b, ej):
        return True
    # order-insensitive check: same intra-system complete-graph edge set
    if a.size != ei.size:
        return False
    key = a.astype(np.int64) * (N_SYS * N_ELEC) + b.astype(np.int64)
    kref = ei.astype(np.int64) * (N_SYS * N_ELEC) + ej.astype(np.int64)
    return np.array_equal(np.sort(key), np.sort(kref))


def _reference_np(h_one, W_qkv, W_out, ln1_scale, ln1_bias, W_mlp, b_mlp,
                  ln2_scale, ln2_bias, e_e_i, e_e_j):
    """Numpy fallback for arbitrary edge lists (never hit for the real inputs)."""
    h = np.asarray(h_one, np.float64)
    n = h.shape[0]
    qkv = h @ np.asarray(W_qkv, np.float64)
    Q, K, V = np.split(qkv, 3, axis=-1)
    Q = Q.reshape(n, HEADS, HD)
    K = K.reshape(n, HEADS, HD)
    V = V.reshape(n, HEADS, HD)
    ei = np.asarray(e_e_i).ravel()
    ej = np.asarray(e_e_j).ravel()
    A = np.einsum("ehd,ehd->eh", Q[ei], K[ej]) / np.sqrt(HD)
    mx = np.full((n, HEADS), -np.inf)
    np.maximum.at(mx, ej, A)
    e = np.exp(A - mx[ej])
    den = np.zeros((n, HEADS))
    np.add.at(den, ej, e)
    P = e / den[ej]
    attn = np.zeros((n, HEADS, HD))
    np.add.at(attn, ei, P[..., None] * V[ej])
    attn = attn.reshape(n, DIM)
    hh = h + attn @ np.asarray(W_out, np.float64)

    def ln(x, s, b):
        mu = x.mean(-1, keepdims=True)
        var = ((x - mu) ** 2).mean(-1, keepdims=True)
        return (x - mu) / np.sqrt(var + EPS) * np.asarray(s, np.float64) + np.asarray(b, np.float64)

    hh = ln(hh, ln1_scale, ln1_bias)
    m = hh @ np.asarray(W_mlp, np.float64) + np.asarray(b_mlp, np.float64)
    hh = hh + m / (1.0 + np.exp(-m))
    hh = ln(hh, ln2_scale, ln2_bias)
    return hh.astype(np.float32)


def _build(flags):
    """Build (and cache) the Bass program. flags = (ln1_aff, ln2_aff, mlp_bias, big_dt)."""
    if flags in _BUILD_CACHE:
        return _BUILD_CACHE[flags]

    import concourse.bass as bass
    import concourse.mybir as mybir
    import concourse.tile as tile
    from concourse import bacc
    from concourse.masks import make_identity

    ln1_aff, ln2_aff, mlp_bias, big_dt = flags
    f32 = mybir.dt.float32
    mm_dt = mybir.dt.float32r if big_dt == "f32r" else mybir.dt.float32
    PS = bass.MemorySpace.PSUM

    def mmcast(ap):
        return ap.bitcast(mm_dt) if big_dt == "f32r" else ap

    nc = bacc.Bacc("TRN2", target_bir_lowering=False, debug=False,
                   num_devices=NCORES)

    h_d = nc.dram_tensor("h", [R, DIM], f32, kind="ExternalInput")
    wq_d = nc.dram_tensor("wq", [DIM, 3 * DIM], f32, kind="ExternalInput")
    wo_d = nc.dram_tensor("wo", [DIM, DIM], f32, kind="ExternalInput")
    wm_d = nc.dram_tensor("wm", [DIM, DIM], f32, kind="ExternalInput")
    ln1s_d = ln1b_d = ln2s_d = ln2b_d = bm_d = None
    if ln1_aff:
        ln1s_d = nc.dram_tensor("ln1s", [DIM], f32, kind="ExternalInput")
        ln1b_d = nc.dram_tensor("ln1b", [DIM], f32, kind="ExternalInput")
    if ln2_aff:
        ln2s_d = nc.dram_tensor("ln2s", [DIM], f32, kind="ExternalInput")
        ln2b_d = nc.dram_tensor("ln2b", [DIM], f32, kind="ExternalInput")
    if mlp_bias:
        bm_d = nc.dram_tensor("bm", [DIM], f32, kind="ExternalInput")
    out_d = nc.dram_tensor("out", [R, DIM], f32, kind="ExternalOutput")

    Exp = mybir.ActivationFunctionType.Exp
    Silu = mybir.ActivationFunctionType.Silu
    Sqrt = mybir.ActivationFunctionType.Sqrt
    SUB = mybir.AluOpType.subtract
    MUL = mybir.AluOpType.mult
    X = mybir.AxisListType.X

    with tile.TileContext(nc) as tc:
        with (
            tc.tile_pool(name="per", bufs=1) as per,          # persistent sbuf
            tc.tile_pool(name="pat", bufs=1, space=PS) as pat,  # persistent psum (attnT)
            tc.tile_pool(name="pst", bufs=3, space=PS) as pst,  # transpose psum
            tc.tile_pool(name="psqk", bufs=2, space=PS) as psqk,
            tc.tile_pool(name="psv", bufs=2, space=PS) as psv,
            tc.tile_pool(name="psS", bufs=4, space=PS) as psS,
            tc.tile_pool(name="psh2", bufs=2, space=PS) as psh2,
            tc.tile_pool(name="psm", bufs=2, space=PS) as psm,
            tc.tile_pool(name="rot", bufs=2) as rot,          # rotating sbuf
            tc.tile_pool(name="rot3", bufs=3) as rot3,
            tc.tile_pool(name="small", bufs=4) as small,
        ):
            # ---- persistent SBUF ----
            ident = per.tile([128, 128], f32, tag="ident")
            make_identity(nc, ident)
            wq = per.tile([128, 2, 3 * DIM], f32, tag="wq")
            wo = per.tile([128, 2, DIM], f32, tag="wo")
            wm = per.tile([128, 2, DIM], f32, tag="wm")
            hsb = per.tile([128, NBLK, DIM], f32, tag="hsb")
            hT = per.tile([128, 2, R], f32, tag="hT")
            QT = per.tile([128, 2, R], f32, tag="QT")
            KT = per.tile([128, 2, R], f32, tag="KT")
            Vn = per.tile([128, NPAIR, DIM], f32, tag="Vn")
            aT = per.tile([128, 2, R], f32, tag="aT")  # attnT sbuf

            nc.sync.dma_start(out=wq, in_=wq_d[:].rearrange("(c p) n -> p c n", p=128))
            nc.sync.dma_start(out=wo, in_=wo_d[:].rearrange("(c p) n -> p c n", p=128))
            nc.sync.dma_start(out=wm, in_=wm_d[:].rearrange("(c p) n -> p c n", p=128))
            nc.sync.dma_start(out=hsb, in_=h_d[:].rearrange("(n p) d -> p n d", p=128))

            if ln1_aff:
                ln1s = per.tile([128, DIM], f32, tag="ln1s")
                ln1b = per.tile([128, DIM], f32, tag="ln1b")
                nc.sync.dma_start(out=ln1s, in_=ln1s_d[:].to_broadcast([128, DIM]))
                nc.sync.dma_start(out=ln1b, in_=ln1b_d[:].to_broadcast([128, DIM]))
            if ln2_aff:
                ln2s = per.tile([128, DIM], f32, tag="ln2s")
                ln2b = per.tile([128, DIM], f32, tag="ln2b")
                nc.sync.dma_start(out=ln2s, in_=ln2s_d[:].to_broadcast([128, DIM]))
                nc.sync.dma_start(out=ln2b, in_=ln2b_d[:].to_broadcast([128, DIM]))
            if mlp_bias:
                bm = per.tile([128, DIM], f32, tag="bm")
                nc.sync.dma_start(out=bm, in_=bm_d[:].to_broadcast([128, DIM]))

            # ---- h -> hT (PE transposes) ----
            for n in range(NBLK):
                for c in range(2):
                    tp = pst.tile([128, 128], f32, tag="tp")
                    nc.tensor.transpose(tp, hsb[:, n, 128 * c:128 * (c + 1)], ident)
                    eng = nc.vector if c == 0 else nc.scalar
                    if c == 0:
                        nc.vector.tensor_copy(out=hT[:, c, 128 * n:128 * (n + 1)], in_=tp)
                    else:
                        nc.scalar.copy(out=hT[:, c, 128 * n:128 * (n + 1)], in_=tp)

            # ---- qkvT: QT, KT (transposed Q/K) ----
            # feature chunks t: 0,1 -> QT chunks; 2,3 -> KT chunks
            for t in range(4):
                ps = psqk.tile([128, R], f32, tag="psqk")
                for k in range(2):
                    nc.tensor.matmul(
                        ps,
                        mmcast(wq[:, k, 128 * t:128 * (t + 1)]),
                        mmcast(hT[:, k, :]),
                        start=(k == 0), stop=(k == 1),
                    )
                dst = QT if t < 2 else KT
                nc.vector.tensor_copy(out=dst[:, t % 2, :], in_=ps)

            # ---- V natural: Vn[j, d] per pair ----
            for q in range(NPAIR):
                ps = psv.tile([128, DIM], f32, tag="psv")
                for k in range(2):
                    nc.tensor.matmul(
                        ps,
                        mmcast(hT[:, k, 128 * q:128 * (q + 1)]),
                        mmcast(wq[:, k, 2 * DIM:3 * DIM]),
                        start=(k == 0), stop=(k == 1),
                    )
                nc.vector.tensor_copy(out=Vn[:, q, :], in_=ps)

            # ---- attnT psum accumulators (persist over all pairs) ----
            at_ps = [[pat.tile([128, NPAIR * 64], f32, tag=f"at{c}{p}")
                      for p in range(2)] for c in range(2)]

            # ---- attention, per system pair ----
            for q in range(NPAIR):
                # scores: S'[j, i] per (head, parity) -> 4 psum banks
                sp = [psS.tile([128, 128], f32, tag="sp") for _ in range(4)]
                for ch in range(2):
                    for hh in range(4):
                        for par in range(2):
                            col = 64 * (2 * q + par)
                            nc.tensor.matmul(
                                sp[hh][64 * par:64 * (par + 1), 64 * ch:64 * (ch + 1)],
                                KT[:, ch, :][32 * hh:32 * (hh + 1), col:col + 64],
                                QT[:, ch, :][32 * hh:32 * (hh + 1), col:col + 64],
                                tile_position=(32 * hh, 64 * par),
                                start=True, stop=True,
                            )
                # exp (scaled); E cols are head-major: head h at 64*h
                E = rot.tile([128, 8 * 64], f32, tag="E")
                Ev = E[:].rearrange("p (h i) -> p h i", i=64)
                for hh in range(4):
                    nc.scalar.activation(
                        out=Ev[:, hh::4, :],
                        in_=sp[hh][:].rearrange("p (c i) -> p c i", i=64),
                        func=Exp, scale=SCALE,
                    )
                # denominators per (j, head) and 1/denominator
                Dn = small.tile([128, 8], f32, tag="Dn")
                nc.vector.reduce_sum(out=Dn, in_=Ev, axis=X)
                Rc = small.tile([128, 8], f32, tag="Rc")
                nc.vector.reciprocal(out=Rc, in_=Dn)
                # V' = V * (1/D) broadcast per head
                Vp = rot.tile([128, DIM], f32, tag="Vp")
                nc.vector.tensor_mul(
                    Vp[:].rearrange("p (h d) -> p h d", d=HD),
                    Vn[:, q, :].rearrange("p (h d) -> p h d", d=HD),
                    Rc[:].to_broadcast([128, 8, HD]),
                )
                # attn^T[d, i] += V'^T E  (per head/parity)
                for ch in range(2):
                    for hh in range(4):
                        hg = 4 * ch + hh
                        for par in range(2):
                            nc.tensor.matmul(
                                at_ps[ch][par][32 * hh:32 * (hh + 1), 64 * q:64 * (q + 1)],
                                Vp[64 * par:64 * (par + 1), 32 * hg:32 * (hg + 1)],
                                E[64 * par:64 * (par + 1), 64 * hg:64 * (hg + 1)],
                                tile_position=(64 * par, 32 * hh),
                                start=True, stop=True,
                            )

            # ---- attnT psum -> sbuf (interleave parities into device order) ----
            for c in range(2):
                av = aT[:, c, :].rearrange("p (q s e) -> p q s e", s=2, e=64)
                nc.vector.tensor_copy(
                    out=av[:, :, 0, :],
                    in_=at_ps[c][0][:].rearrange("p (q e) -> p q e", e=64))
                nc.scalar.copy(
                    out=av[:, :, 1, :],
                    in_=at_ps[c][1][:].rearrange("p (q e) -> p q e", e=64))

            # ---- per 128-row block: W_out, residual, LN1, MLP, LN2, store ----
            for n in range(NBLK):
                ps2 = psh2.tile([128, DIM], f32, tag="ps2")
                for c in range(2):
                    nc.tensor.matmul(
                        ps2,
                        mmcast(aT[:, c, 128 * n:128 * (n + 1)]),
                        mmcast(wo[:, c, :]),
                        start=(c == 0), stop=(c == 1),
                    )
                r1 = rot3.tile([128, DIM], f32, tag="r1")
                nc.vector.tensor_add(r1, hsb[:, n, :], ps2)

                # LN1
                st = small.tile([128, 6], f32, tag="st")
                nc.vector.bn_stats(out=st, in_=r1)
                mv = small.tile([128, 2], f32, tag="mv")
                nc.vector.bn_aggr(out=mv, in_=st)
                sd = small.tile([128, 1], f32, tag="sd")
                nc.scalar.activation(out=sd, in_=mv[:, 1:2], func=Sqrt, bias=EPS)
                rs = small.tile([128, 1], f32, tag="rs")
                nc.vector.reciprocal(out=rs, in_=sd)
                h3 = rot3.tile([128, DIM], f32, tag="h3")
                nc.vector.tensor_scalar(h3, r1, mv[:, 0:1], rs, op0=SUB, op1=MUL)
                if ln1_aff:
                    nc.vector.tensor_mul(h3, h3, ln1s)
                    nc.vector.tensor_add(h3, h3, ln1b)

                # h3 block -> h3T columns
                h3t = rot.tile([128, 2, 128], f32, tag="h3t")
                for c in range(2):
                    tp = pst.tile([128, 128], f32, tag="tp")
                    nc.tensor.transpose(tp, h3[:, 128 * c:128 * (c + 1)], ident)
                    if c == 0:
                        nc.vector.tensor_copy(out=h3t[:, c, :], in_=tp)
                    else:
                        nc.scalar.copy(out=h3t[:, c, :], in_=tp)

                # MLP
                psm_t = psm.tile([128, DIM], f32, tag="psm")
                for c in range(2):
                    nc.tensor.matmul(
                        psm_t,
                        mmcast(h3t[:, c, :]),
                        mmcast(wm[:, c, :]),
                        start=(c == 0), stop=(c == 1),
                    )
                if mlp_bias:
                    nc.vector.tensor_add(psm_t, psm_t, bm)
                sl = rot.tile([128, DIM], f32, tag="sl")
                nc.scalar.activation(out=sl, in_=psm_t, func=Silu)
                h4 = rot3.tile([128, DIM], f32, tag="h4")
                nc.vector.tensor_add(h4, h3, sl)

                # LN2
                st2 = small.tile([128, 6], f32, tag="st2")
                nc.vector.bn_stats(out=st2, in_=h4)
                mv2 = small.tile([128, 2], f32, tag="mv2")
                nc.vector.bn_aggr(out=mv2, in_=st2)
                sd2 = small.tile([128, 1], f32, tag="sd2")
                nc.scalar.activation(out=sd2, in_=mv2[:, 1:2], func=Sqrt, bias=EPS)
                rs2 = small.tile([128, 1], f32, tag="rs2")
                nc.vector.reciprocal(out=rs2, in_=sd2)
                ot = rot3.tile([128, DIM], f32, tag="ot")
                nc.vector.tensor_scalar(ot, h4, mv2[:, 0:1], rs2, op0=SUB, op1=MUL)
                if ln2_aff:
                    nc.vector.tensor_mul(ot, ot, ln2s)
                    nc.vector.tensor_add(ot, ot, ln2b)

                nc.sync.dma_start(out=out_d[128 * n:128 * (n + 1), :], in_=ot)

    nc.compile()
    _BUILD_CACHE[flags] = nc
    return nc


def kernel(h_one, W_qkv, W_out, ln1_scale, ln1_bias, W_mlp, b_mlp,
           ln2_scale, ln2_bias, e_e_i, e_e_j, _trace=False):
    h_one = np.ascontiguousarray(np.asarray(h_one, np.float32))
    W_qkv = np.ascontiguousarray(np.asarray(W_qkv, np.float32))
    W_out = np.ascontiguousarray(np.asarray(W_out, np.float32))
    W_mlp = np.ascontiguousarray(np.asarray(W_mlp, np.float32))
    ln1_scale = np.asarray(ln1_scale, np.float32)
    ln1_bias = np.asarray(ln1_bias, np.float32)
    ln2_scale = np.asarray(ln2_scale, np.float32)
    ln2_bias = np.asarray(ln2_bias, np.float32)
    b_mlp = np.asarray(b_mlp, np.float32)

    if not _edges_are_blockdense(e_e_i, e_e_j):
        return _reference_np(h_one, W_qkv, W_out, ln1_scale, ln1_bias, W_mlp,
                             b_mlp, ln2_scale, ln2_bias, e_e_i, e_e_j)

    ln1_aff = not (np.all(ln1_scale == 1.0) and np.all(ln1_bias == 0.0))
    ln2_aff = not (np.all(ln2_scale == 1.0) and np.all(ln2_bias == 0.0))
    mlp_bias = not np.all(b_mlp == 0.0)
    nc = _build((ln1_aff, ln2_aff, mlp_bias, BIG_MM_DTYPE))

    from concourse.bass_utils import run_bass_kernel_spmd

    in_maps = []
    for c in range(NCORES):
        m = {
            "h": h_one[R * c:R * (c + 1)],
            "wq": W_qkv,
            "wo": W_out,
            "wm": W_mlp,
        }
        if ln1_aff:
            m["ln1s"] = ln1_scale
            m["ln1b"] = ln1_bias
        if ln2_aff:
            m["ln2s"] = ln2_scale
            m["ln2b"] = ln2_bias
        if mlp_bias:
            m["bm"] = b_mlp
        in_maps.append(m)

    res = run_bass_kernel_spmd(nc, in_maps, core_ids=list(range(NCORES)),
                               trace=_trace)
    out = np.concatenate([res.results[c]["out"] for c in range(NCORES)], axis=0)
    if _trace:
        kernel._last_results = res
    return out


# revision 11
# speedup vs baseline: 12.2266x; 12.2266x over previous
"""Trainium2 Bass kernel for nn_Attention_15899968929956.

Block-diagonal GNN message passing == dense per-system attention:
64 systems x 64 electrons, DIM=256, 8 heads x head_dim 32. Edges are all
intra-system pairs, so per (system, head):
  S'[j, i] = K[j] . Q[i] / sqrt(hd)           (j, i in [0, 64))
  P[j, i]  = exp(S') / sum_i' exp(S'[j, i'])  (softmax segmented by key j)
  attn[i]  = sum_j P[j, i] * V[j]
then out = LN2(h3 + silu(h3 @ W_mlp + b)), h3 = LN1(h + attn @ W_out).

Sharding: 8 systems (512 electrons) per NeuronCore, parameters replicated.

Layouts per core (all SBUF tiles 128 partitions):
  hsb  [128, 4, 256]   natural rows (block n = device rows 128n..128n+128)
  hT   [128, 2, 512]   h transposed (chunk c = features 128c.., col = row idx)
  QT/KT[128, 2, 512]   transposed Q/K (chunk c = heads 4c..4c+3, 32 rows each)
  Vn   [128, 4, 256]   V natural, pair q = rows 128q..
  aT   [128, 2, 512]   attn transposed (chunk c = heads 4c.., col = row idx)
Small matmuls use PE array tiling: scores K=32/M=64 (8 concurrent tiles),
PV K=64/M=32 (8 concurrent tiles). Big matmuls use float32r (full PE rate).
"""

import sys

if "/opt/trn_rl_repo" not in sys.path:
    sys.path.insert(0, "/opt/trn_rl_repo")

from contextlib import ExitStack

import numpy as np

N_SYS = 64
N_ELEC = 64
DIM = 256
HEADS = 8
HD = DIM // HEADS  # 32
EPS = 1e-6
NCORES = 8
SPC = N_SYS // NCORES      # systems per core = 8
R = SPC * N_ELEC           # rows per core = 512
NPAIR = SPC // 2           # system pairs per core = 4
NBLK = R // 128            # 128-row blocks per core = 4
SCALE = 1.0 / float(np.sqrt(HD))

# "f32" (exact) or "f32r" (reduced-precision multiplies on the big matmuls,
# full PE rate at N>=256 instead of 1/4 rate for fp32)
BIG_MM_DTYPE = "f32r"

_BUILD_CACHE: dict = {}


def _expected_edges():
    ii, jj = np.meshgrid(np.arange(N_ELEC), np.arange(N_ELEC), indexing="ij")
    offs = (np.arange(N_SYS) * N_ELEC)[:, None, None]
    ei = (offs + ii[None]).reshape(-1).astype(np.int32)
    ej = (offs + jj[None]).reshape(-1).astype(np.int32)
    return ei, ej


def _edges_are_blockdense(e_e_i, e_e_j):
    ei, ej = _expected_edges()
    a = np.asarray(e_e_i).ravel()
    b = np.asarray(e_e_j).ravel()
    if a.shape != ei.shape or b.shape != ej.shape:
        return False
    if np.array_equal(a, ei) and np.array_equal(b, ej):
        return True
    key = a.astype(np.int64) * (N_SYS * N_ELEC) + b.astype(np.int64)
    kref = ei.astype(np.int64) * (N_SYS * N_ELEC) + ej.astype(np.int64)
    return np.array_equal(np.sort(key), np.sort(kref))


def _reference_np(h_one, W_qkv, W_out, ln1_scale, ln1_bias, W_mlp, b_mlp,
                  ln2_scale, ln2_bias, e_e_i, e_e_j):
    """Numpy fallback for arbitrary edge lists (never hit for the real inputs)."""
    h = np.asarray(h_one, np.float64)
    n = h.shape[0]
    qkv = h @ np.asarray(W_qkv, np.float64)
    Q, K, V = np.split(qkv, 3, axis=-1)
    Q = Q.reshape(n, HEADS, HD)
    K = K.reshape(n, HEADS, HD)
    V = V.reshape(n, HEADS, HD)
    ei = np.asarray(e_e_i).ravel()
    ej = np.asarray(e_e_j).ravel()
    A = np.einsum("ehd,ehd->eh", Q[ei], K[ej]) / np.sqrt(HD)
    mx = np.full((n, HEADS), -np.inf)
    np.maximum.at(mx, ej, A)
    e = np.exp(A - mx[ej])
    den = np.zeros((n, HEADS))
    np.add.at(den, ej, e)
    P = e / den[ej]
    attn = np.zeros((n, HEADS, HD))
    np.add.at(attn, ei, P[..., None] * V[ej])
    attn = attn.reshape(n, DIM)
    hh = h + attn @ np.asarray(W_out, np.float64)

    def ln(x, s, b):
        mu = x.mean(-1, keepdims=True)
        var = ((x - mu) ** 2).mean(-1, keepdims=True)
        return (x - mu) / np.sqrt(var + EPS) * np.asarray(s, np.float64) \
            + np.asarray(b, np.float64)

    hh = ln(hh, ln1_scale, ln1_bias)
    m = hh @ np.asarray(W_mlp, np.float64) + np.asarray(b_mlp, np.float64)
    hh = hh + m / (1.0 + np.exp(-m))
    hh = ln(hh, ln2_scale, ln2_bias)
    return hh.astype(np.float32)


def _build(flags, chain=1):
    """Build + compile the Bass program.

    flags = (ln1_aff, ln2_aff, mlp_bias, big_dt). chain>1 repeats the whole
    body, iteration t reading h from the out tensor written by t-1 (timing
    harness: marginal iteration == steady-state kernel incl. all DMA).
    """
    key = (flags, chain)
    if key in _BUILD_CACHE:
        return _BUILD_CACHE[key]

    import concourse.bass as bass
    import concourse.mybir as mybir
    import concourse.tile as tile
    from concourse import bacc
    from concourse.masks import make_identity

    ln1_aff, ln2_aff, mlp_bias, big_dt = flags
    f32 = mybir.dt.float32
    mdt = mybir.dt.float32r if big_dt == "f32r" else mybir.dt.float32
    PS = bass.MemorySpace.PSUM

    nc = bacc.Bacc("TRN2", target_bir_lowering=False, debug=False,
                   num_devices=NCORES)

    h_d = nc.dram_tensor("h", [R, DIM], f32, kind="ExternalInput")
    wq_d = nc.dram_tensor("wq", [DIM, 3 * DIM], mdt, kind="ExternalInput")
    wo_d = nc.dram_tensor("wo", [DIM, DIM], mdt, kind="ExternalInput")
    wm_d = nc.dram_tensor("wm", [DIM, DIM], mdt, kind="ExternalInput")
    if ln1_aff:
        ln1s_d = nc.dram_tensor("ln1s", [DIM], f32, kind="ExternalInput")
        ln1b_d = nc.dram_tensor("ln1b", [DIM], f32, kind="ExternalInput")
    if ln2_aff:
        ln2s_d = nc.dram_tensor("ln2s", [DIM], f32, kind="ExternalInput")
        ln2b_d = nc.dram_tensor("ln2b", [DIM], f32, kind="ExternalInput")
    if mlp_bias:
        bm_d = nc.dram_tensor("bm", [DIM], f32, kind="ExternalInput")
    out_d = nc.dram_tensor("out", [R, DIM], f32, kind="ExternalOutput")

    Exp = mybir.ActivationFunctionType.Exp
    Silu = mybir.ActivationFunctionType.Silu
    Sqrt = mybir.ActivationFunctionType.Sqrt
    SUB = mybir.AluOpType.subtract
    MUL = mybir.AluOpType.mult
    X = mybir.AxisListType.X

    with tile.TileContext(nc) as tc:
        with (
            tc.tile_pool(name="per", bufs=1) as per,    # persistent sbuf
            tc.tile_pool(name="rot", bufs=2) as rot,    # rotating sbuf
            tc.tile_pool(name="rot3", bufs=3) as rot3,
            tc.tile_pool(name="small", bufs=4) as small,
        ):
            # ---- persistent SBUF ----
            ident = per.tile([128, 128], f32, tag="ident")
            make_identity(nc, ident)
            epst = per.tile([128, 1], f32, tag="epst")
            nc.vector.memset(epst, EPS)
            zt = per.tile([128, 1], f32, tag="zt")
            nc.vector.memset(zt, 0.0)
            wq = per.tile([128, 2, 3 * DIM], mdt, tag="wq")
            wo = per.tile([128, 2, DIM], mdt, tag="wo")
            wm = per.tile([128, 2, DIM], mdt, tag="wm")
            hsb = per.tile([128, NBLK, DIM], f32, tag="hsb")
            hT = per.tile([128, 2, R], mdt, tag="hT")
            QT = per.tile([128, 2, R], f32, tag="QT")
            KT = per.tile([128, 2, R], f32, tag="KT")
            Vn = per.tile([128, NPAIR, DIM], f32, tag="Vn")
            aT = per.tile([128, 2, R], mdt, tag="aT")  # attnT sbuf
            if ln1_aff:
                ln1s = per.tile([128, DIM], f32, tag="ln1s")
                ln1b = per.tile([128, DIM], f32, tag="ln1b")
            if ln2_aff:
                ln2s = per.tile([128, DIM], f32, tag="ln2s")
                ln2b = per.tile([128, DIM], f32, tag="ln2b")
            if mlp_bias:
                bm = per.tile([128, DIM], f32, tag="bm")

            for it in range(chain):
                h_src = h_d if it == 0 else out_d

                nc.sync.dma_start(
                    out=wq, in_=wq_d[:].rearrange("(c p) n -> p c n", p=128))
                nc.sync.dma_start(
                    out=wo, in_=wo_d[:].rearrange("(c p) n -> p c n", p=128))
                nc.sync.dma_start(
                    out=wm, in_=wm_d[:].rearrange("(c p) n -> p c n", p=128))
                nc.sync.dma_start(
                    out=hsb, in_=h_src[:].rearrange("(n p) d -> p n d", p=128))
                if ln1_aff:
                    nc.sync.dma_start(out=ln1s, in_=ln1s_d[:].to_broadcast([128, DIM]))
                    nc.sync.dma_start(out=ln1b, in_=ln1b_d[:].to_broadcast([128, DIM]))
                if ln2_aff:
                    nc.sync.dma_start(out=ln2s, in_=ln2s_d[:].to_broadcast([128, DIM]))
                    nc.sync.dma_start(out=ln2b, in_=ln2b_d[:].to_broadcast([128, DIM]))
                if mlp_bias:
                    nc.sync.dma_start(out=bm, in_=bm_d[:].to_broadcast([128, DIM]))

                # ---- phase A: h -> hT transposes, QKV projections ----
                pa = ExitStack()
                pst = pa.enter_context(
                    tc.tile_pool(name=f"pst{it}", bufs=3, space=PS))
                psqk = pa.enter_context(
                    tc.tile_pool(name=f"psqk{it}", bufs=2, space=PS))
                psv = pa.enter_context(
                    tc.tile_pool(name=f"psv{it}", bufs=2, space=PS))
                for n in range(NBLK):
                    for c in range(2):
                        tp = pst.tile([128, 128], f32, tag="tp")
                        nc.tensor.transpose(tp, hsb[:, n, 128 * c:128 * (c + 1)], ident)
                        if c == 0:
                            nc.vector.tensor_copy(
                                out=hT[:, c, 128 * n:128 * (n + 1)], in_=tp)
                        else:
                            nc.scalar.copy(
                                out=hT[:, c, 128 * n:128 * (n + 1)], in_=tp)

                # qkvT: feature chunks t: 0,1 -> QT; 2,3 -> KT
                for t in range(4):
                    ps = psqk.tile([128, R], f32, tag="psqk")
                    for k in range(2):
                        nc.tensor.matmul(
                            ps,
                            wq[:, k, 128 * t:128 * (t + 1)],
                            hT[:, k, :],
                            start=(k == 0), stop=(k == 1),
                        )
                    dst = QT if t < 2 else KT
                    nc.vector.tensor_copy(out=dst[:, t % 2, :], in_=ps)

                # V natural per pair
                for q in range(NPAIR):
                    ps = psv.tile([128, DIM], f32, tag="psv")
                    for k in range(2):
                        nc.tensor.matmul(
                            ps,
                            hT[:, k, 128 * q:128 * (q + 1)],
                            wq[:, k, 2 * DIM:3 * DIM],
                            start=(k == 0), stop=(k == 1),
                        )
                    nc.vector.tensor_copy(out=Vn[:, q, :], in_=ps)

                pa.close()
                # ---- phase B: attention ----
                pb = ExitStack()
                pat = pb.enter_context(
                    tc.tile_pool(name=f"pat{it}", bufs=1, space=PS))
                psS = pb.enter_context(
                    tc.tile_pool(name=f"psS{it}", bufs=4, space=PS))
                at_ps = [[pat.tile([128, NPAIR * 64], f32, tag=f"at{c}{p}",
                                   name=f"at_ps{c}{p}")
                          for p in range(2)] for c in range(2)]

                for q in range(NPAIR):
                    # scores: bank hh <- head hh (cols 0:64) + head hh+4 (64:128)
                    sp = [psS.tile([128, 128], f32, tag="sp", name=f"sp{q}_{b}")
                          for b in range(4)]
                    for ch in range(2):
                        for hh in range(4):
                            for par in range(2):
                                col = 64 * (2 * q + par)
                                nc.tensor.matmul(
                                    sp[hh][64 * par:64 * (par + 1),
                                           64 * ch:64 * (ch + 1)],
                                    KT[:, ch, :][32 * hh:32 * (hh + 1), col:col + 64],
                                    QT[:, ch, :][32 * hh:32 * (hh + 1), col:col + 64],
                                    tile_position=(32 * hh, 64 * par),
                                    start=True, stop=True,
                                )
                    # exp (scaled); E cols head-major (head h at 64h)
                    E = rot.tile([128, 8 * 64], f32, tag="E")
                    Ev = E[:].rearrange("p (h i) -> p h i", i=64)
                    Ecv = E[:].rearrange("p (c h i) -> p c h i", c=2, i=64)
                    for hh in range(4):
                        nc.scalar.activation(
                            out=Ecv[:, :, hh, :],
                            in_=sp[hh][:].rearrange("p (c i) -> p c i", i=64),
                            func=Exp, bias=zt, scale=SCALE,
                        )
                    Dn = small.tile([128, 8], f32, tag="Dn")
                    nc.vector.reduce_sum(out=Dn, in_=Ev, axis=X)
                    Rc = small.tile([128, 8], f32, tag="Rc")
                    nc.vector.reciprocal(out=Rc, in_=Dn)
                    # V' = V * (1/D), broadcast per head
                    Vp = rot.tile([128, DIM], f32, tag="Vp")
                    nc.vector.tensor_mul(
                        Vp[:].rearrange("p (h d) -> p h d", d=HD),
                        Vn[:, q, :].rearrange("p (h d) -> p h d", d=HD),
                        Rc[:].to_broadcast([128, 8, HD]),
                    )
                    # attn^T[d, i] = sum_j V'[j, d] E[j, i]
                    for ch in range(2):
                        for hh in range(4):
                            hg = 4 * ch + hh
                            for par in range(2):
                                nc.tensor.matmul(
                                    at_ps[ch][par][32 * hh:32 * (hh + 1),
                                                   64 * q:64 * (q + 1)],
                                    Vp[64 * par:64 * (par + 1),
                                       32 * hg:32 * (hg + 1)],
                                    E[64 * par:64 * (par + 1),
                                      64 * hg:64 * (hg + 1)],
                                    tile_position=(64 * par, 32 * hh),
                                    start=True, stop=True,
                                )

                # attnT psum -> sbuf (interleave parities into device order)
                for c in range(2):
                    av = aT[:, c, :].rearrange("p (q s e) -> p q s e", s=2, e=64)
                    nc.vector.tensor_copy(
                        out=av[:, :, 0, :],
                        in_=at_ps[c][0][:].rearrange("p (q e) -> p q e", e=64))
                    nc.scalar.copy(
                        out=av[:, :, 1, :],
                        in_=at_ps[c][1][:].rearrange("p (q e) -> p q e", e=64))

                pb.close()
                # ---- phase C: W_out, residual, LN1, MLP, LN2, store ----
                pc = ExitStack()
                psh2 = pc.enter_context(
                    tc.tile_pool(name=f"psh2{it}", bufs=2, space=PS))
                psm = pc.enter_context(
                    tc.tile_pool(name=f"psm{it}", bufs=2, space=PS))
                pst = pc.enter_context(
                    tc.tile_pool(name=f"pst2{it}", bufs=3, space=PS))
                for n in range(NBLK):
                    ps2 = psh2.tile([128, DIM], f32, tag="ps2")
                    for c in range(2):
                        nc.tensor.matmul(
                            ps2,
                            aT[:, c, 128 * n:128 * (n + 1)],
                            wo[:, c, :],
                            start=(c == 0), stop=(c == 1),
                        )
                    r1 = rot3.tile([128, DIM], f32, tag="r1")
                    nc.vector.tensor_add(r1, hsb[:, n, :], ps2)

                    # LN1
                    st = small.tile([128, 6], f32, tag="st")
                    nc.vector.bn_stats(out=st, in_=r1)
                    mv = small.tile([128, 2], f32, tag="mv")
                    nc.vector.bn_aggr(out=mv, in_=st)
                    sd = small.tile([128, 1], f32, tag="sd")
                    nc.scalar.activation(out=sd, in_=mv[:, 1:2], func=Sqrt, bias=epst)
                    rs = small.tile([128, 1], f32, tag="rs")
                    nc.vector.reciprocal(out=rs, in_=sd)
                    h3 = rot3.tile([128, DIM], f32, tag="h3")
                    nc.vector.tensor_scalar(h3, r1, mv[:, 0:1], rs, op0=SUB, op1=MUL)
                    if ln1_aff:
                        nc.vector.tensor_mul(h3, h3, ln1s)
                        nc.vector.tensor_add(h3, h3, ln1b)

                    # h3 block -> h3T columns
                    h3t = rot.tile([128, 2, 128], mdt, tag="h3t")
                    for c in range(2):
                        tp = pst.tile([128, 128], f32, tag="tp")
                        nc.tensor.transpose(tp, h3[:, 128 * c:128 * (c + 1)], ident)
                        if c == 0:
                            nc.vector.tensor_copy(out=h3t[:, c, :], in_=tp)
                        else:
                            nc.scalar.copy(out=h3t[:, c, :], in_=tp)

                    # MLP
                    psm_t = psm.tile([128, DIM], f32, tag="psm")
                    for c in range(2):
                        nc.tensor.matmul(
                            psm_t,
                            h3t[:, c, :],
                            wm[:, c, :],
                            start=(c == 0), stop=(c == 1),
                        )
                    if mlp_bias:
                        nc.vector.tensor_add(psm_t, psm_t, bm)
                    sl = rot.tile([128, DIM], f32, tag="sl")
                    nc.scalar.activation(out=sl, in_=psm_t, func=Silu, bias=zt)
                    h4 = rot3.tile([128, DIM], f32, tag="h4")
                    nc.vector.tensor_add(h4, h3, sl)

                    # LN2
                    st2 = small.tile([128, 6], f32, tag="st2")
                    nc.vector.bn_stats(out=st2, in_=h4)
                    mv2 = small.tile([128, 2], f32, tag="mv2")
                    nc.vector.bn_aggr(out=mv2, in_=st2)
                    sd2 = small.tile([128, 1], f32, tag="sd2")
                    nc.scalar.activation(out=sd2, in_=mv2[:, 1:2], func=Sqrt,
                                         bias=epst)
                    rs2 = small.tile([128, 1], f32, tag="rs2")
                    nc.vector.reciprocal(out=rs2, in_=sd2)
                    ot = rot3.tile([128, DIM], f32, tag="ot")
                    nc.vector.tensor_scalar(ot, h4, mv2[:, 0:1], rs2,
                                            op0=SUB, op1=MUL)
                    if ln2_aff:
                        nc.vector.tensor_mul(ot, ot, ln2s)
                        nc.vector.tensor_add(ot, ot, ln2b)

                    nc.sync.dma_start(out=out_d[128 * n:128 * (n + 1), :], in_=ot)

                pc.close()

    nc.compile()
    _BUILD_CACHE[key] = nc
    return nc


def kernel(h_one, W_qkv, W_out, ln1_scale, ln1_bias, W_mlp, b_mlp,
           ln2_scale, ln2_bias, e_e_i, e_e_j, _trace=False, _chain=1):
    h_one = np.ascontiguousarray(np.asarray(h_one, np.float32))
    W_qkv = np.ascontiguousarray(np.asarray(W_qkv, np.float32))
    W_out = np.ascontiguousarray(np.asarray(W_out, np.float32))
    W_mlp = np.ascontiguousarray(np.asarray(W_mlp, np.float32))
    ln1_scale = np.asarray(ln1_scale, np.float32)
    ln1_bias = np.asarray(ln1_bias, np.float32)
    ln2_scale = np.asarray(ln2_scale, np.float32)
    ln2_bias = np.asarray(ln2_bias, np.float32)
    b_mlp = np.asarray(b_mlp, np.float32)

    if not _edges_are_blockdense(e_e_i, e_e_j):
        return _reference_np(h_one, W_qkv, W_out, ln1_scale, ln1_bias, W_mlp,
                             b_mlp, ln2_scale, ln2_bias, e_e_i, e_e_j)

    ln1_aff = not (np.all(ln1_scale == 1.0) and np.all(ln1_bias == 0.0))
    ln2_aff = not (np.all(ln2_scale == 1.0) and np.all(ln2_bias == 0.0))
    mlp_bias = not np.all(b_mlp == 0.0)
    nc = _build((ln1_aff, ln2_aff, mlp_bias, BIG_MM_DTYPE), chain=_chain)

    from concourse.bass_utils import run_bass_kernel_spmd

    in_maps = []
    for c in range(NCORES):
        m = {
            "h": h_one[R * c:R * (c + 1)],
            "wq": W_qkv,
            "wo": W_out,
            "wm": W_mlp,
        }
        if ln1_aff:
            m["ln1s"] = ln1_scale
            m["ln1b"] = ln1_bias
        if ln2_aff:
            m["ln2s"] = ln2_scale
            m["ln2b"] = ln2_bias
        if mlp_bias:
            m["bm"] = b_mlp
        in_maps.append(m)

    res = run_bass_kernel_spmd(nc, in_maps, core_ids=list(range(NCORES)),
                               trace=_trace)
    out = np.concatenate([res.results[c]["out"] for c in range(NCORES)], axis=0)
    if _trace:
        kernel._last_results = res
    return out
